# revision 31
# baseline (speedup 1.0000x reference)
"""Causal-attention (QKV projection + softmax(QK^T/sqrt(d))V) on 8 trn2 cores.

Contract: kernel(x, Wq, Wk, Wv) takes FULL inputs
  x [4, 4096, 768] f32, Wq/Wk/Wv [768, 128] f32
and returns the FULL output [4, 4096, 128] f32.

Sharding: 2 cores per batch. Core with parity h in {0,1} of batch b owns query
rows h::2 (perfect causal load balance). The host permutes the per-core input
to xT_p = concat(x[b, h::2], x[b, 1-h::2]).T so one compiled SPMD program runs
on every core; causality of the permuted key order is enforced by 0/1
multiplicative masks applied to the exp'd diagonal score tiles on the DVE
(keeps the tensor engine free of mask matmuls).

Per-core device program (fp16 matmuls, fp32 PSUM accumulation):
  input x DMA'd in consumption-ordered column phases on one HWDGE ring
  (phases time-share the core's single DMA-fabric port, so ordering them is
  what makes q-tile 0's data land first); 26 warm-up matmuls bridge the DMA
  window so the PE's HAM clock-gate reaches 2.4GHz and never re-throttles.
  K^T[d=128, S], Q^T[d=128, S/2], V[key-tile][128 keys, 128 d] projections;
  per 512-query tile: scores^T tiles [128 keys, 512 q] -> exp(scale*s - 1.5)
  on ScalarE (no max subtraction: scores ~ N(0,1); the bias cancels in the
  num/den division) -> AV accumulated in PSUM. Exp-sums accumulate in two
  parallel DVE chains per tile so the end-of-kernel serial add->DMA tail is
  halved; the first chain ships mid-tile.
  Outputs: numerator OUT^T [128, S/2] f16 and exp-sum blocks [128, 4*S/2]
  f16; the host reduces exp-sums to denominators, divides, and scatters.
"""
import numpy as np

import concourse.bass as bass
import concourse.mybir as mybir
import concourse.tile as tile_mod
from concourse.tile import ScopedClock, VectorClock
from concourse.tile_sem_assignment import N_PROCS
from concourse.bass_utils import run_bass_kernel_spmd

f32 = mybir.dt.float32
f16 = mybir.dt.float16
f8 = mybir.dt.float8e4

B, S, D_IN, D = 4, 4096, 768, 128
N_DIN = D_IN // 128  # 6
TQ = 512             # queries per q-tile
SCALE = 1.0 / np.sqrt(np.float32(D))
EBIAS = -1.5         # exp centering bias; cancels in the num/den division
AF = mybir.ActivationFunctionType

# ---------------------------------------------------------------------------
# Workarounds: the walrus build in this container accepts only ONE sync-wait
# command per instruction. TileContext's exit drain carries one wait per
# active proc, and Tile's sem assignment emits multi-wait instructions.
# Split both onto single-wait carrier instructions.
# ---------------------------------------------------------------------------


def _split_drain_and_barrier(self, tick_clock, wait_clock):
    gc = tick_clock.global_clock
    for p in range(N_PROCS):
        if gc[p] == 0:
            continue
        vc = VectorClock([gc[q] if q == p else 0 for q in range(N_PROCS)])
        d = self.nc.sync.drain()
        wait_clock.add_sem_waits(d.ins, ScopedClock({None: vc}))
    self.nc.all_engine_barrier()
    assert self.sems is not None
    popped = self.nc._tile_sem_poison_stack.pop()
    assert popped is self._sem_poison
    self.nc.clear_and_free_semaphores(list(self.sems.allocated().values()))
    self.nc.all_engine_barrier()


tile_mod.TileContext._drain_and_barrier = _split_drain_and_barrier


def _split_waits(nc, max_waits=1):
    for fn in nc.m.functions:
        for bb in fn.blocks:
            insts = bb.instructions
            if not any(
                i.sync_info and i.sync_info.on_wait
                and len(i.sync_info.on_wait) > max_waits
                for i in insts
            ):
                continue
            new = []
            for inst in insts:
                si = inst.sync_info
                ow = list(si.on_wait) if si and si.on_wait else []
                if len(ow) > max_waits:
                    excess, keep = ow[:-max_waits], ow[-max_waits:]
                    for j, w in enumerate(excess):
                        new.append(
                            mybir.InstEventSemaphore(
                                name=f"{inst.name}-wsplit{j}",
                                engine=inst.engine,
                                ins=[],
                                outs=[],
                                sync_info=mybir.SyncInfo(
                                    on_wait=[w], on_update=[]
                                ),
                            )
                        )
                    inst.sync_info = mybir.SyncInfo(
                        on_wait=keep, on_update=list(si.on_update or [])
                    )
                new.append(inst)
            bb.instructions = new


# ---------------------------------------------------------------------------
# Device program
# ---------------------------------------------------------------------------


def _build():
    NQ = S // 2
    n_qt = NQ // TQ
    n_kt_half = NQ // 128

    nc = bass.Bass()
    xT = nc.declare_dram_parameter("xT", [D_IN, S], f16, isOutput=False)
    W = nc.declare_dram_parameter("W", [128, N_DIN * 3 * D], f16, isOutput=False)
    mask = nc.declare_dram_parameter("mask", [128, 8 * TQ], f16, isOutput=False)
    out_num = nc.declare_dram_parameter("out_num", [D, NQ], f16, isOutput=True)
    out_den = nc.declare_dram_parameter("out_den", [128, 4 * NQ], f16, isOutput=True)

    with tile_mod.TileContext(nc) as tc:
        with (
            tc.tile_pool(name="persist", bufs=1) as persist,
            tc.tile_pool(name="work", bufs=6) as work,
            tc.tile_pool(name="sacc_p", bufs=4) as sacc_p,
            tc.tile_pool(name="outp", bufs=2) as outp,
            tc.tile_pool(name="ps_big", bufs=2, space="PSUM") as ps_big,
            tc.tile_pool(name="ps_out", bufs=2, space="PSUM") as ps_out,
            tc.tile_pool(name="ps_sml", bufs=2, space="PSUM") as ps_sml,
        ):
            x_sb = [persist.tile([128, S], f16, tag=f"x{di}", name=f"x{di}")
                    for di in range(N_DIN)]
            w_all = persist.tile([128, N_DIN * 3 * D], f16, tag="w_all")
            m_all = persist.tile([128, 8 * TQ], f16, tag="m_all")
            kt_sb = [persist.tile([128, 512], f16, tag=f"kt{c}", name=f"kt{c}")
                     for c in range(S // 512)]
            qt_sb = [persist.tile([128, TQ], f16, tag=f"qt{t}", name=f"qt{t}")
                     for t in range(n_qt)]
            v_sb = [persist.tile([128, D], f16, tag=f"v{k}", name=f"v{k}")
                    for k in range(2 * n_kt_half)]

            w_sb = [w_all[:, 3 * D * di:3 * D * (di + 1)] for di in range(N_DIN)]
            m_sb = [m_all[:, TQ * r:TQ * (r + 1)] for r in range(8)]

            # input DMAs: the DMA fabric bandwidth (~380GB/s/core) is shared
            # across all rings, so phases must be TIME-serialized, not
            # ring-separated. One ring (sync, fast HWDGE path), per-queue
            # FIFO preserves phase order: W, then x in consumption order
            # (own-half cols 0:512 first so projections start earliest).
            def x_views(di):
                src_v = xT[128 * di:128 * (di + 1), :].rearrange(
                    "p (b c) -> p b c", b=2)
                dst_v = x_sb[di].rearrange("p (b c) -> p b c", b=2)
                return src_v, dst_v

            w_src = W.rearrange("p (di c) -> p di c", di=N_DIN)
            w_dst = w_all.rearrange("p (di c) -> p di c", di=N_DIN)
            nc.sync.dma_start(out=w_dst[:, :, D:3 * D], in_=w_src[:, :, D:3 * D])
            phases = [
                ((0, 1), 0, 512),    # A1: own half, q-tile 0 cols
                ((1, 2), 0, 512),    # A2: other half, q-tile 0 cols
                ((0, 1), 512, 1024),   # B1: own half, q-tile 1 cols
                ((1, 2), 512, 1024),   # B2: other half, q-tile 1 cols
                ((0, 2), 1024, 1536),  # C1: q-tile 2 cols
                ((0, 2), 1536, 2048),  # C2: q-tile 3 cols
            ]
            for pi, ((b0, b1), lo, hi) in enumerate(phases):
                for di in range(N_DIN):
                    src_v, dst_v = x_views(di)
                    nc.sync.dma_start(
                        out=dst_v[:, b0:b1, lo:hi],
                        in_=src_v[:, b0:b1, lo:hi],
                    )
                if pi == 0:
                    nc.sync.dma_start(out=w_dst[:, :, 0:D], in_=w_src[:, :, 0:D])
                if pi == 1:
                    nc.sync.dma_start(out=m_all[:], in_=mask[:])
            # PE pre-warm during the input-DMA wait: HAM un-throttles after
            # ~3.4us of sustained activity, so the first real matmuls run at
            # 2.4GHz instead of 1.2GHz. 26 matmuls bridge ~7us until phase-A
            # x data has landed, so the PE never re-throttles.
            warm_sb = persist.tile([128, 512], f16, tag="warm")
            nc.vector.memset(warm_sb[:], 0.0)
            bias_sb = persist.tile([128, 1], f32, tag="ebias")
            nc.vector.memset(bias_sb[:], EBIAS)
            psw = ps_sml.tile([128, 512], f32, tag="sml", name="warm_ps")
            for _ in range(26):
                nc.tensor.matmul(
                    psw[:], lhsT=warm_sb[:, 0:128], rhs=warm_sb[:],
                    start=True, stop=True,
                )

            def project_kt(c):
                ps = ps_sml.tile([128, 512], f32, tag="sml", name=f"pkt{c}")
                for di in range(N_DIN):
                    nc.tensor.matmul(
                        ps[:],
                        lhsT=w_sb[di][:, D:2 * D],
                        rhs=x_sb[di][:, 512 * c:512 * (c + 1)],
                        start=(di == 0),
                        stop=(di == N_DIN - 1),
                    )
                nc.scalar.activation(kt_sb[c][:], ps[:], AF.Copy)

            def project_qt(t):
                ps = ps_sml.tile([128, 512], f32, tag="sml", name=f"pqt{t}")
                for di in range(N_DIN):
                    nc.tensor.matmul(
                        ps[:],
                        lhsT=w_sb[di][:, 0:D],
                        rhs=x_sb[di][:, TQ * t:TQ * (t + 1)],
                        start=(di == 0),
                        stop=(di == N_DIN - 1),
                    )
                nc.scalar.activation(qt_sb[t][:], ps[:], AF.Copy)

            def project_v_chunk(c):
                for k in range(4 * c, 4 * c + 4):
                    ps = ps_sml.tile([128, D], f32, tag="sml", name=f"pv{k}")
                    for di in range(N_DIN):
                        nc.tensor.matmul(
                            ps[:],
                            lhsT=x_sb[di][:, 128 * k:128 * (k + 1)],
                            rhs=w_sb[di][:, 2 * D:3 * D],
                            start=(di == 0),
                            stop=(di == N_DIN - 1),
                        )
                    nc.vector.tensor_copy(v_sb[k][:], ps[:])

            def proj_groups(T):
                return [
                    lambda: project_kt(T),
                    lambda: project_v_chunk(T),
                    lambda: project_qt(T),
                    lambda: project_kt(n_qt + T),
                    lambda: project_v_chunk(n_qt + T),
                ]

            def make_state(t):
                own = [2 * j for j in range(2 * (t + 1))]
                oth = [n_kt_half + p for p in own]
                if t == n_qt - 1:
                    # end the kernel on a non-diagonal pair: the final
                    # exp->mask->add->DMA tail chain is shorter
                    oth = [oth[0]] + oth[-2:] + oth[1:-2]
                pairs = own + oth
                L = len(pairs)
                return dict(
                    t=t, pairs=pairs, L=L, n_av=2 * L, av_i=0,
                    po=ps_out.tile([128, TQ], f32, tag="out", name=f"po{t}"),
                    bounds=[0, L // 2, L], blk0=2 * t,
                    saccs=[
                        sacc_p.tile([128, 2 * TQ], f16, tag="sacc",
                                    name=f"sacc{t}_{c}")
                        for c in range(2)
                    ],
                )

            def do_pair(st, i):
                t, po = st["t"], st["po"]
                kp = st["pairs"][i]
                L, n_av, bounds = st["L"], st["n_av"], st["bounds"]
                ps = ps_big.tile([128, 2 * TQ], f32, tag="big",
                                 name=f"s{t}_{kp}")
                pt = work.tile([128, 2 * TQ], f16, tag="pt",
                               name=f"p{t}_{kp}")
                half2 = kp >= n_kt_half
                rel = kp - n_kt_half if half2 else kp
                diag = 4 * t <= rel < 4 * t + 4
                # heavy-diag pairs (r_off=2: cols < 256/384 fully masked)
                # are sliced end-to-end: scores, exp, mask and sacc only
                # touch the live region, so no engine pays for masked
                # columns and no instruction reads unwritten bytes
                bpair = diag and rel - 4 * t == 2
                for s_ in (0, 1):
                    kt = kp + s_
                    lo = 128 * (rel - 4 * t + s_) if bpair else 0
                    nc.tensor.matmul(
                        ps[:, TQ * s_ + lo:TQ * (s_ + 1)],
                        lhsT=kt_sb[kt // 4][:, 128 * (kt % 4):128 * (kt % 4 + 1)],
                        rhs=qt_sb[t][:, lo:],
                        start=True,
                        stop=True,
                    )
                if bpair:
                    for s_ in (0, 1):
                        lo = 128 * (rel - 4 * t + s_)
                        nc.scalar.activation(
                            pt[:, TQ * s_ + lo:TQ * (s_ + 1)],
                            ps[:, TQ * s_ + lo:TQ * (s_ + 1)],
                            AF.Exp, scale=float(SCALE), bias=bias_sb[:])
                else:
                    nc.scalar.activation(pt[:], ps[:], AF.Exp,
                                         scale=float(SCALE), bias=bias_sb[:])
                if diag:
                    # multiplicative 0/1 causal mask on DVE (keeps the
                    # tensor engine free of mask matmuls)
                    for s_ in (0, 1):
                        r0 = (4 if half2 else 0) + rel - 4 * t + s_
                        lo = 128 * (rel - 4 * t + s_) if bpair else 0
                        nc.vector.tensor_mul(
                            pt[:, TQ * s_ + lo:TQ * (s_ + 1)],
                            pt[:, TQ * s_ + lo:TQ * (s_ + 1)],
                            m_sb[r0][:, lo:],
                        )
                for s_ in (0, 1):
                    kt = kp + s_
                    lo_q = 128 * (rel - 4 * t + s_) if diag else 0
                    nc.tensor.matmul(
                        po[:, lo_q:TQ],
                        lhsT=v_sb[kt][:],
                        rhs=pt[:, TQ * s_ + lo_q:TQ * (s_ + 1)],
                        start=(st["av_i"] == 0),
                        stop=(st["av_i"] == n_av - 1),
                    )
                    st["av_i"] += 1
                ci = sum(1 for b in bounds[1:] if i >= b)
                sacc = st["saccs"][ci]
                if i in bounds:
                    assert not bpair, "chain init must be a fully-exp'd pair"
                    nc.vector.tensor_copy(sacc[:], pt[:])
                elif bpair:
                    for s_ in (0, 1):
                        lo = 128 * (rel - 4 * t + s_)
                        sl = slice(TQ * s_ + lo, TQ * (s_ + 1))
                        nc.vector.tensor_add(sacc[:, sl], sacc[:, sl],
                                             pt[:, sl])
                else:
                    nc.vector.tensor_add(sacc[:], sacc[:], pt[:])
                if i + 1 in bounds[1:]:
                    nc.sync.dma_start(
                        out=out_den[:, 2 * TQ * (st["blk0"] + ci):
                                    2 * TQ * (st["blk0"] + ci + 1)],
                        in_=sacc[:],
                    )
                if i == L - 1:
                    ob = outp.tile([128, TQ], f16, tag="ob", name=f"ob{t}")
                    nc.scalar.activation(ob[:], po[:], AF.Copy)
                    nc.sync.dma_start(out=out_num[:, TQ * t:TQ * (t + 1)],
                                      in_=ob[:])

            # schedule: tiles 0 and 1 alone (their x phases gate them);
            # tile 2's projections pipelined into tile 1's late pairs;
            # tile 3's projections ahead of the merged block, then tiles
            # 2 and 3 pair-interleaved so the PE always has an independent
            # pair to run while the other tile's AV waits on its exp
            for g in proj_groups(0):
                g()
            st = make_state(0)
            for i in range(st["L"]):
                do_pair(st, i)
            for g in proj_groups(1):
                g()
            st = make_state(1)
            ins1 = {st["L"] - 5 + gi: g
                    for gi, g in enumerate(proj_groups(2))}
            for i in range(st["L"]):
                if i in ins1:
                    ins1[i]()
                do_pair(st, i)
            for g in proj_groups(3):
                g()
            st2, st3 = make_state(2), make_state(3)
            for i in range(st2["L"]):
                do_pair(st2, i)
                do_pair(st3, i)
            for i in range(st2["L"], st3["L"]):
                do_pair(st3, i)
            assert st2["av_i"] == st2["n_av"]
            assert st3["av_i"] == st3["n_av"]
    _split_waits(nc)
    return nc


_NC_CACHE = []


def _get_nc():
    if not _NC_CACHE:
        _NC_CACHE.append(_build())
    return _NC_CACHE[0]


def _host_inputs(x, Wq, Wk, Wv):
    W3 = np.concatenate([Wq, Wk, Wv], axis=1).astype(np.float16)  # [768, 384]
    W = np.ascontiguousarray(
        W3.reshape(N_DIN, 128, 3 * D).transpose(1, 0, 2).reshape(128, N_DIN * 3 * D)
    )
    u = np.arange(128)[:, None]
    i = np.arange(TQ)[None, :]
    masks = {}
    for h in (0, 1):
        m = np.zeros((8, 128, TQ), np.float32)
        for r in range(4):
            m[r] = (128 * r + u <= i)
            m[4 + r] = (128 * r + u <= i - 1 + h)
        flat = m.transpose(1, 0, 2).reshape(128, 8 * TQ)
        masks[h] = np.ascontiguousarray(flat).astype(np.float16)
    in_maps = []
    for c in range(2 * B):
        b, h = divmod(c, 2)
        xp = np.concatenate([x[b, h::2], x[b, 1 - h::2]], axis=0)  # [S, 768]
        xT_p = np.ascontiguousarray(xp.T.astype(np.float16))  # [768, S]
        in_maps.append({"xT": xT_p, "W": W, "mask": masks[h]})
    return in_maps


def kernel(x, Wq, Wk, Wv):
    x = np.asarray(x, np.float32)
    Wq = np.asarray(Wq, np.float32)
    Wk = np.asarray(Wk, np.float32)
    Wv = np.asarray(Wv, np.float32)
    nc = _get_nc()
    in_maps = _host_inputs(x, Wq, Wk, Wv)
    res = run_bass_kernel_spmd(nc, in_maps, list(range(2 * B)))
    out = np.empty((B, S, D), np.float32)
    NQ = S // 2
    for c in range(2 * B):
        b, h = divmod(c, 2)
        num = res.results[c]["out_num"].astype(np.float32)  # [128, NQ] f16
        sacc = res.results[c]["out_den"].astype(np.float32)  # [128, 4*NQ]
        s3 = sacc.reshape(128, NQ // TQ, 4, TQ)
        den = s3.sum(axis=(0, 2)).reshape(NQ)
        out[b, h::2, :] = (num / den[None, :]).T
    return out



# revision 33
# speedup vs baseline: 1.0524x; 1.0524x over previous
"""Causal-attention (QKV projection + softmax(QK^T/sqrt(d))V) on 8 trn2 cores.

Contract: kernel(x, Wq, Wk, Wv) takes FULL inputs
  x [4, 4096, 768] f32, Wq/Wk/Wv [768, 128] f32
and returns the FULL output [4, 4096, 128] f32.

Sharding: 2 cores per batch. Core with parity h in {0,1} of batch b owns query
rows h::2 (perfect causal load balance). The host permutes the per-core input
to xT_p = concat(x[b, h::2], x[b, 1-h::2]).T so one compiled SPMD program runs
on every core; causality of the permuted key order is enforced by 0/1
multiplicative masks applied to the exp'd diagonal score tiles on the DVE
(keeps the tensor engine free of mask matmuls).

Per-core device program (fp16 matmuls, fp32 PSUM accumulation):
  input x DMA'd in consumption-ordered column phases on one HWDGE ring
  (phases time-share the core's single DMA-fabric port, so ordering them is
  what makes q-tile 0's data land first); 26 warm-up matmuls bridge the DMA
  window so the PE's HAM clock-gate reaches 2.4GHz and never re-throttles.
  K^T[d=128, S], Q^T[d=128, S/2], V[key-tile][128 keys, 128 d] projections;
  per 512-query tile: scores^T tiles [128 keys, 512 q] -> exp(scale*s - 1.5)
  on ScalarE (no max subtraction: scores ~ N(0,1); the bias cancels in the
  num/den division) -> AV accumulated in PSUM. Exp-sums accumulate in two
  parallel DVE chains per tile so the end-of-kernel serial add->DMA tail is
  halved; the first chain ships mid-tile.
  Outputs: numerator OUT^T [128, S/2] f16 and exp-sum blocks [128, 4*S/2]
  f16; the host reduces exp-sums to denominators, divides, and scatters.
"""
import numpy as np

import concourse.bass as bass
import concourse.mybir as mybir
import concourse.tile as tile_mod
from concourse.tile import ScopedClock, VectorClock
from concourse.tile_sem_assignment import N_PROCS
from concourse.bass_utils import run_bass_kernel_spmd

f32 = mybir.dt.float32
f16 = mybir.dt.float16
f8 = mybir.dt.float8e4

B, S, D_IN, D = 4, 4096, 768, 128
N_DIN = D_IN // 128  # 6
TQ = 512             # queries per q-tile
SCALE = 1.0 / np.sqrt(np.float32(D))
EBIAS = -1.5         # exp centering bias; cancels in the num/den division
AF = mybir.ActivationFunctionType

# ---------------------------------------------------------------------------
# Workarounds: the walrus build in this container accepts only ONE sync-wait
# command per instruction. TileContext's exit drain carries one wait per
# active proc, and Tile's sem assignment emits multi-wait instructions.
# Split both onto single-wait carrier instructions.
# ---------------------------------------------------------------------------


def _split_drain_and_barrier(self, tick_clock, wait_clock):
    gc = tick_clock.global_clock
    for p in range(N_PROCS):
        if gc[p] == 0:
            continue
        vc = VectorClock([gc[q] if q == p else 0 for q in range(N_PROCS)])
        d = self.nc.sync.drain()
        wait_clock.add_sem_waits(d.ins, ScopedClock({None: vc}))
    self.nc.all_engine_barrier()
    assert self.sems is not None
    popped = self.nc._tile_sem_poison_stack.pop()
    assert popped is self._sem_poison
    self.nc.clear_and_free_semaphores(list(self.sems.allocated().values()))
    self.nc.all_engine_barrier()


tile_mod.TileContext._drain_and_barrier = _split_drain_and_barrier


def _split_waits(nc, max_waits=1):
    for fn in nc.m.functions:
        for bb in fn.blocks:
            insts = bb.instructions
            if not any(
                i.sync_info and i.sync_info.on_wait
                and len(i.sync_info.on_wait) > max_waits
                for i in insts
            ):
                continue
            new = []
            for inst in insts:
                si = inst.sync_info
                ow = list(si.on_wait) if si and si.on_wait else []
                if len(ow) > max_waits:
                    excess, keep = ow[:-max_waits], ow[-max_waits:]
                    for j, w in enumerate(excess):
                        new.append(
                            mybir.InstEventSemaphore(
                                name=f"{inst.name}-wsplit{j}",
                                engine=inst.engine,
                                ins=[],
                                outs=[],
                                sync_info=mybir.SyncInfo(
                                    on_wait=[w], on_update=[]
                                ),
                            )
                        )
                    inst.sync_info = mybir.SyncInfo(
                        on_wait=keep, on_update=list(si.on_update or [])
                    )
                new.append(inst)
            bb.instructions = new


# ---------------------------------------------------------------------------
# Device program
# ---------------------------------------------------------------------------


def _build():
    NQ = S // 2
    n_qt = NQ // TQ
    n_kt_half = NQ // 128

    nc = bass.Bass()
    xT = nc.declare_dram_parameter("xT", [D_IN, S], f16, isOutput=False)
    W = nc.declare_dram_parameter("W", [128, N_DIN * 3 * D], f16, isOutput=False)
    mask = nc.declare_dram_parameter("mask", [128, 8 * TQ], f16, isOutput=False)
    out_num = nc.declare_dram_parameter("out_num", [D, NQ], f16, isOutput=True)
    out_den = nc.declare_dram_parameter("out_den", [128, 4 * NQ], f16, isOutput=True)

    with tile_mod.TileContext(nc) as tc:
        with (
            tc.tile_pool(name="persist", bufs=1) as persist,
            tc.tile_pool(name="work", bufs=6) as work,
            tc.tile_pool(name="sacc_p", bufs=4) as sacc_p,
            tc.tile_pool(name="outp", bufs=2) as outp,
            tc.tile_pool(name="ps_big", bufs=2, space="PSUM") as ps_big,
            tc.tile_pool(name="ps_out", bufs=2, space="PSUM") as ps_out,
            tc.tile_pool(name="ps_sml", bufs=2, space="PSUM") as ps_sml,
        ):
            x_sb = [persist.tile([128, S], f16, tag=f"x{di}", name=f"x{di}")
                    for di in range(N_DIN)]
            w_all = persist.tile([128, N_DIN * 3 * D], f16, tag="w_all")
            m_all = persist.tile([128, 8 * TQ], f16, tag="m_all")
            kt_sb = [persist.tile([128, 512], f16, tag=f"kt{c}", name=f"kt{c}")
                     for c in range(S // 512)]
            qt_sb = [persist.tile([128, TQ], f16, tag=f"qt{t}", name=f"qt{t}")
                     for t in range(n_qt)]
            v_sb = [persist.tile([128, D], f16, tag=f"v{k}", name=f"v{k}")
                    for k in range(2 * n_kt_half)]

            w_sb = [w_all[:, 3 * D * di:3 * D * (di + 1)] for di in range(N_DIN)]
            m_sb = [m_all[:, TQ * r:TQ * (r + 1)] for r in range(8)]

            # input DMAs: the DMA fabric bandwidth (~380GB/s/core) is shared
            # across all rings, so phases must be TIME-serialized, not
            # ring-separated. One ring (sync, fast HWDGE path), per-queue
            # FIFO preserves phase order: W, then x in consumption order
            # (own-half cols 0:512 first so projections start earliest).
            def x_views(di):
                src_v = xT[128 * di:128 * (di + 1), :].rearrange(
                    "p (b c) -> p b c", b=2)
                dst_v = x_sb[di].rearrange("p (b c) -> p b c", b=2)
                return src_v, dst_v

            w_src = W.rearrange("p (di c) -> p di c", di=N_DIN)
            w_dst = w_all.rearrange("p (di c) -> p di c", di=N_DIN)
            nc.sync.dma_start(out=w_dst[:, :, D:3 * D], in_=w_src[:, :, D:3 * D])
            phases = [
                ((0, 1), 0, 512),    # A1: own half, q-tile 0 cols
                ((1, 2), 0, 512),    # A2: other half, q-tile 0 cols
                ((0, 1), 512, 1024),   # B1: own half, q-tile 1 cols
                ((1, 2), 512, 1024),   # B2: other half, q-tile 1 cols
                ((0, 2), 1024, 1536),  # C1: q-tile 2 cols
                ((0, 2), 1536, 2048),  # C2: q-tile 3 cols
            ]
            for pi, ((b0, b1), lo, hi) in enumerate(phases):
                for di in range(N_DIN):
                    src_v, dst_v = x_views(di)
                    nc.sync.dma_start(
                        out=dst_v[:, b0:b1, lo:hi],
                        in_=src_v[:, b0:b1, lo:hi],
                    )
                if pi == 0:
                    nc.sync.dma_start(out=w_dst[:, :, 0:D], in_=w_src[:, :, 0:D])
                if pi == 1:
                    nc.sync.dma_start(out=m_all[:], in_=mask[:])
            # PE pre-warm during the input-DMA wait: HAM un-throttles after
            # ~3.4us of sustained activity, so the first real matmuls run at
            # 2.4GHz instead of 1.2GHz. 26 matmuls bridge ~7us until phase-A
            # x data has landed, so the PE never re-throttles.
            warm_sb = persist.tile([128, 512], f16, tag="warm")
            nc.vector.memset(warm_sb[:], 0.0)
            bias_sb = persist.tile([128, 1], f32, tag="ebias")
            nc.vector.memset(bias_sb[:], EBIAS)
            psw = ps_sml.tile([128, 512], f32, tag="sml", name="warm_ps")
            for _ in range(26):
                nc.tensor.matmul(
                    psw[:], lhsT=warm_sb[:, 0:128], rhs=warm_sb[:],
                    start=True, stop=True,
                )

            def project_kt(c):
                ps = ps_sml.tile([128, 512], f32, tag="sml", name=f"pkt{c}")
                for di in range(N_DIN):
                    nc.tensor.matmul(
                        ps[:],
                        lhsT=w_sb[di][:, D:2 * D],
                        rhs=x_sb[di][:, 512 * c:512 * (c + 1)],
                        start=(di == 0),
                        stop=(di == N_DIN - 1),
                    )
                nc.scalar.activation(kt_sb[c][:], ps[:], AF.Copy)

            def project_qt(t):
                ps = ps_sml.tile([128, 512], f32, tag="sml", name=f"pqt{t}")
                for di in range(N_DIN):
                    nc.tensor.matmul(
                        ps[:],
                        lhsT=w_sb[di][:, 0:D],
                        rhs=x_sb[di][:, TQ * t:TQ * (t + 1)],
                        start=(di == 0),
                        stop=(di == N_DIN - 1),
                    )
                nc.scalar.activation(qt_sb[t][:], ps[:], AF.Copy)

            def project_v_chunk(c):
                for k in range(4 * c, 4 * c + 4):
                    ps = ps_sml.tile([128, D], f32, tag="sml", name=f"pv{k}")
                    for di in range(N_DIN):
                        nc.tensor.matmul(
                            ps[:],
                            lhsT=x_sb[di][:, 128 * k:128 * (k + 1)],
                            rhs=w_sb[di][:, 2 * D:3 * D],
                            start=(di == 0),
                            stop=(di == N_DIN - 1),
                        )
                    nc.vector.tensor_copy(v_sb[k][:], ps[:])

            def proj_groups(T):
                return [
                    lambda: project_kt(T),
                    lambda: project_v_chunk(T),
                    lambda: project_qt(T),
                    lambda: project_kt(n_qt + T),
                    lambda: project_v_chunk(n_qt + T),
                ]

            for t in range(n_qt):
                # projections for tiles 0/1 at the head (their x phases gate
                # them anyway); tiles 2/3 are software-pipelined into the
                # previous tile's late pairs below, so the PE never idles at
                # the tile boundary waiting for the last exps
                if t == 0:
                    for g in proj_groups(0):
                        g()
                elif t == 1:
                    # tile 1's own-half groups were pipelined into tile 0's
                    # last pair (they need only phase B1); the other-half
                    # groups (phase B2) run here
                    for g in proj_groups(1)[3:]:
                        g()

                po = ps_out.tile([128, TQ], f32, tag="out", name=f"po{t}")
                own = [2 * j for j in range(2 * (t + 1))]
                oth = [n_kt_half + p for p in own]
                if t == n_qt - 1:
                    # end the kernel on a non-diagonal pair: the final
                    # exp->mask->add->DMA tail chain is shorter
                    oth = [oth[0]] + oth[-2:] + oth[1:-2]
                pairs = own + oth
                L = len(pairs)
                n_av = 2 * L
                # exp-sums accumulate in two parallel chains per tile so
                # the end-of-kernel serial add->DMA tail is halved; the first
                # chain ships mid-tile. Host sums all blocks.
                bounds = [0, L // 2, L]
                blk0 = 2 * t
                saccs = [
                    sacc_p.tile([128, 2 * TQ], f16, tag="sacc",
                                name=f"sacc{t}_{c}")
                    for c in range(len(bounds) - 1)
                ]
                inserts = {}
                if t == 0:
                    inserts[L - 1] = proj_groups(1)[:3]
                elif t in (1, 2):
                    inserts = {L - 5 + gi: [g]
                               for gi, g in enumerate(proj_groups(t + 1))}
                for i, kp in enumerate(pairs):
                    for g in inserts.get(i, ()):
                        g()
                    ps = ps_big.tile([128, 2 * TQ], f32, tag="big",
                                     name=f"s{t}_{kp}")
                    pt = work.tile([128, 2 * TQ], f16, tag="pt",
                                   name=f"p{t}_{kp}")
                    half2 = kp >= n_kt_half
                    rel = kp - n_kt_half if half2 else kp
                    diag = 4 * t <= rel < 4 * t + 4
                    # heavy-diag pairs (r_off=2: cols < 256/384 fully masked)
                    # are sliced end-to-end: scores, exp, mask and sacc only
                    # touch the live region, so no engine pays for masked
                    # columns and no instruction reads unwritten bytes
                    bpair = diag and rel - 4 * t == 2
                    for s_ in (0, 1):
                        kt = kp + s_
                        lo = 128 * (rel - 4 * t + s_) if bpair else 0
                        nc.tensor.matmul(
                            ps[:, TQ * s_ + lo:TQ * (s_ + 1)],
                            lhsT=kt_sb[kt // 4][:, 128 * (kt % 4):128 * (kt % 4 + 1)],
                            rhs=qt_sb[t][:, lo:],
                            start=True,
                            stop=True,
                        )
                    if bpair:
                        for s_ in (0, 1):
                            lo = 128 * (rel - 4 * t + s_)
                            nc.scalar.activation(
                                pt[:, TQ * s_ + lo:TQ * (s_ + 1)],
                                ps[:, TQ * s_ + lo:TQ * (s_ + 1)],
                                AF.Exp, scale=float(SCALE), bias=bias_sb[:])
                    else:
                        nc.scalar.activation(pt[:], ps[:], AF.Exp,
                                             scale=float(SCALE), bias=bias_sb[:])
                    if diag:
                        # multiplicative 0/1 causal mask on DVE (keeps the
                        # tensor engine free of mask matmuls)
                        for s_ in (0, 1):
                            r0 = (4 if half2 else 0) + rel - 4 * t + s_
                            lo = 128 * (rel - 4 * t + s_) if bpair else 0
                            nc.vector.tensor_mul(
                                pt[:, TQ * s_ + lo:TQ * (s_ + 1)],
                                pt[:, TQ * s_ + lo:TQ * (s_ + 1)],
                                m_sb[r0][:, lo:],
                            )
                    for s_ in (0, 1):
                        kt = kp + s_
                        lo_q = 128 * (rel - 4 * t + s_) if diag else 0
                        nc.tensor.matmul(
                            po[:, lo_q:TQ],
                            lhsT=v_sb[kt][:],
                            rhs=pt[:, TQ * s_ + lo_q:TQ * (s_ + 1)],
                            start=(2 * i + s_ == 0),
                            stop=(2 * i + s_ == n_av - 1),
                        )
                    ci = sum(1 for b in bounds[1:] if i >= b)
                    sacc = saccs[ci]
                    if i in bounds:
                        assert not bpair, "chain init must be a fully-exp'd pair"
                        nc.vector.tensor_copy(sacc[:], pt[:])
                    elif bpair:
                        for s_ in (0, 1):
                            lo = 128 * (rel - 4 * t + s_)
                            sl = slice(TQ * s_ + lo, TQ * (s_ + 1))
                            nc.vector.tensor_add(sacc[:, sl], sacc[:, sl],
                                                 pt[:, sl])
                    else:
                        nc.vector.tensor_add(sacc[:], sacc[:], pt[:])
                    if i + 1 in bounds[1:]:
                        nc.sync.dma_start(
                            out=out_den[:, 2 * TQ * (blk0 + ci):
                                        2 * TQ * (blk0 + ci + 1)],
                            in_=sacc[:],
                        )
                ob = outp.tile([128, TQ], f16, tag="ob", name=f"ob{t}")
                nc.scalar.activation(ob[:], po[:], AF.Copy)
                nc.sync.dma_start(out=out_num[:, TQ * t:TQ * (t + 1)], in_=ob[:])
    _split_waits(nc)
    return nc


_NC_CACHE = []


def _get_nc():
    if not _NC_CACHE:
        _NC_CACHE.append(_build())
    return _NC_CACHE[0]


def _host_inputs(x, Wq, Wk, Wv):
    W3 = np.concatenate([Wq, Wk, Wv], axis=1).astype(np.float16)  # [768, 384]
    W = np.ascontiguousarray(
        W3.reshape(N_DIN, 128, 3 * D).transpose(1, 0, 2).reshape(128, N_DIN * 3 * D)
    )
    u = np.arange(128)[:, None]
    i = np.arange(TQ)[None, :]
    masks = {}
    for h in (0, 1):
        m = np.zeros((8, 128, TQ), np.float32)
        for r in range(4):
            m[r] = (128 * r + u <= i)
            m[4 + r] = (128 * r + u <= i - 1 + h)
        flat = m.transpose(1, 0, 2).reshape(128, 8 * TQ)
        masks[h] = np.ascontiguousarray(flat).astype(np.float16)
    in_maps = []
    for c in range(2 * B):
        b, h = divmod(c, 2)
        xp = np.concatenate([x[b, h::2], x[b, 1 - h::2]], axis=0)  # [S, 768]
        xT_p = np.ascontiguousarray(xp.T.astype(np.float16))  # [768, S]
        in_maps.append({"xT": xT_p, "W": W, "mask": masks[h]})
    return in_maps


def kernel(x, Wq, Wk, Wv):
    x = np.asarray(x, np.float32)
    Wq = np.asarray(Wq, np.float32)
    Wk = np.asarray(Wk, np.float32)
    Wv = np.asarray(Wv, np.float32)
    nc = _get_nc()
    in_maps = _host_inputs(x, Wq, Wk, Wv)
    res = run_bass_kernel_spmd(nc, in_maps, list(range(2 * B)))
    out = np.empty((B, S, D), np.float32)
    NQ = S // 2
    for c in range(2 * B):
        b, h = divmod(c, 2)
        num = res.results[c]["out_num"].astype(np.float32)  # [128, NQ] f16
        sacc = res.results[c]["out_den"].astype(np.float32)  # [128, 4*NQ]
        s3 = sacc.reshape(128, NQ // TQ, 4, TQ)
        den = s3.sum(axis=(0, 2)).reshape(NQ)
        out[b, h::2, :] = (num / den[None, :]).T
    return out

